# revision 21
# baseline (speedup 1.0000x reference)
"""DockingScorePredictor Trainium2 kernel — valid-pair compaction.

Data-parallel over complexes: 8 cores, one complex (512 protein x 64 ligand
atoms) per core.  Only pairs within the 8A cutoff (~43%) are processed:
host packs valid pairs into NT tiles of 512 slots, each tile drawing its
protein atoms from a window of <=32 atoms (an atom's pairs may split
across consecutive tiles, so NT = ceil(cnt/512) exactly).

Per tile, ONE K=128 bf16 matmul produces the whole first layer:
  rows  0:32  of lhsT = W1c          x rhs rows  0:32  = radial basis
  rows 32:64  of lhsT = z1_base[win] x rhs rows 32:64  = one-hot protein slot
  rows 64:128 of lhsT = hlWb         x rhs rows 64:128 = one-hot ligand atom
so z1 = z1_base[p] + hlWb[l] + rb@W1c in a single 512-col pass; b1 enters
via the relu1 bias port.  Then z2 = W2.T a1 and z3 = W3.T a2 (f32r):
3 matmuls x 512 cols per tile vs 5 x 512 x 64 dense tiles before.

Engine balance (per tile): PE 3 matmuls; DVE relu1 + relu2-left; ACT
relu2-right + relu3(bias b3, accum).  b1 rides in the ligand one-hot
rows of lhsT (exactly one ligand 1 per valid column), so relu1 is
bias-free.  The radial basis, one-hots, z1_base=hp@W1a and hlWb=hl@W1b
are host-precomputed per-atom/per-pair prep (<1% of the pair-MLP FLOPs,
which all stay on device).  No masking: pad slots (zero one-hots, zero
rb) contribute the constant h_pad = relu-chain(0), folded into br1 on
host.  rhs/lhsT stream in as 4-tile chunked DMAs (big packets),
everything bf16 on the first layer (rel err ~1e-3, tolerance 2e-2).
"""
import numpy as np
from contextlib import ExitStack

import ml_dtypes

import concourse.bass as bass
import concourse.bacc as bacc
import concourse.tile as tile
from concourse import mybir
from concourse import bass_utils

F32 = mybir.dt.float32
F32R = mybir.dt.float32r
BF16 = mybir.dt.bfloat16
AF = mybir.ActivationFunctionType
ALU = mybir.AluOpType

B, P, L = 8, 512, 64
H, RB = 128, 32
CUTOFF = 8.0
N_CORES = 8
CAP = 512                      # pair slots per tile
NPW = 32                       # protein-atom window per tile
CHUNK = 4                      # tiles per DMA chunk
WIDTH = 0.5 * CUTOFF / RB + 1e-8
SPL = 256                      # relu1 column split (DVE gets [0:SPL])

_CACHE = {}


def _build_nc(NT):
    nc = bacc.Bacc("TRN2", target_bir_lowering=False, debug=False,
                   num_devices=N_CORES)
    d = {}
    NCH = (NT + CHUNK - 1) // CHUNK

    def inp(name, shape, dt):
        d[name] = nc.dram_tensor(name, shape, dt, kind="ExternalInput").ap()

    inp("rhs", [H, CAP * CHUNK * NCH], BF16)   # rb rows 0:32, one-hots 32:128
    inp("lhsT", [H, H * CHUNK * NCH], BF16)
    inp("W2", [H, H], BF16)
    inp("W3", [H, H], BF16)
    inp("b2", [H, 1], F32)
    inp("b3", [H, 1], F32)

    acc_ap = nc.dram_tensor("acc", [H, NT], F32, kind="ExternalOutput").ap()

    with tile.TileContext(nc) as tc:
        with ExitStack() as ctx:
            const = ctx.enter_context(tc.tile_pool(name="const", bufs=1))
            rhsP = ctx.enter_context(tc.tile_pool(name="rhsP", bufs=3))
            lhsP = ctx.enter_context(tc.tile_pool(name="lhsP", bufs=3))
            a1P = ctx.enter_context(tc.tile_pool(name="a1P", bufs=3))
            a2P = ctx.enter_context(tc.tile_pool(name="a2P", bufs=3))
            a3P = ctx.enter_context(tc.tile_pool(name="a3P", bufs=3))
            psA = ctx.enter_context(tc.tile_pool(name="psA", bufs=3, space="PSUM"))
            psB = ctx.enter_context(tc.tile_pool(name="psB", bufs=2, space="PSUM"))
            psC = ctx.enter_context(tc.tile_pool(name="psC", bufs=3, space="PSUM"))

            rhs_c, lhs_c = {}, {}
            z1_t, z2_t, z3_t, a1_t, a2_t = {}, {}, {}, {}, {}

            def s_dma(g, split_first=False):
                rhs = rhsP.tile([H, CAP * CHUNK], BF16, tag="rhs", name=f"rhs{g}")
                lhs = lhsP.tile([H, H * CHUNK], BF16, tag="lhs", name=f"lhs{g}")
                base_r, base_l = CAP * CHUNK * g, H * CHUNK * g
                if split_first:
                    # tile 0 slices land first so z1(0) starts ~1us in
                    nc.sync.dma_start(out=rhs[:, 0:CAP],
                                      in_=d["rhs"][:, base_r:base_r + CAP])
                    nc.gpsimd.dma_start(out=lhs[:, 0:H],
                                        in_=d["lhsT"][:, base_l:base_l + H])
                    nc.sync.dma_start(out=rhs[:, CAP:],
                                      in_=d["rhs"][:, base_r + CAP:base_r + CAP * CHUNK])
                    nc.gpsimd.dma_start(out=lhs[:, H:],
                                        in_=d["lhsT"][:, base_l + H:base_l + H * CHUNK])
                else:
                    nc.sync.dma_start(out=rhs[:, :],
                                      in_=d["rhs"][:, base_r:base_r + CAP * CHUNK])
                    nc.gpsimd.dma_start(out=lhs[:, :],
                                        in_=d["lhsT"][:, base_l:base_l + H * CHUNK])
                rhs_c[g], lhs_c[g] = rhs, lhs

            s_dma(0, split_first=True)
            for g0 in (1, 2):
                if g0 < NCH:
                    s_dma(g0)
            t = {}
            for name, shape, dt in [
                ("b2", [H, 1], F32), ("b3", [H, 1], F32),
                ("W2", [H, H], BF16), ("W3", [H, H], BF16),
            ]:
                t[name] = const.tile(shape, dt, tag=name, name=name)
                nc.sync.dma_start(out=t[name], in_=d[name])

            acc = const.tile([H, NT], F32, tag="acc", name="acc")
            zeros = const.tile([H, CAP], F32, tag="zeros", name="zeros")
            nc.vector.memset(zeros[:, :], 0.0)

            # PE warmup: three fp32 matmuls (~1us+ each at cold clock) plus
            # the immediately-following z1 stream span the 3.4us HAM activity
            # window, so real matmuls run at 2.4 GHz from tile 1 on
            for w in range(3):
                wps = psC.tile([H, CAP], F32, tag="z3", name=f"warm{w}")
                nc.tensor.matmul(out=wps[:, :], lhsT=zeros[:, 0:H],
                                 rhs=zeros[:, :], start=True, stop=True)

            def s_z1(j):
                g, s = divmod(j, CHUNK)
                z1 = psA.tile([H, CAP], F32, tag="z1", name=f"z1_{j}")
                nc.tensor.matmul(out=z1[:, :],
                                 lhsT=lhs_c[g][:, H * s:H * (s + 1)],
                                 rhs=rhs_c[g][:, CAP * s:CAP * (s + 1)],
                                 start=True, stop=True)
                z1_t[j] = z1
                if s == CHUNK - 1:
                    rhs_c.pop(g), lhs_c.pop(g)

            def s_relu1(j):
                # b1 folded into lhsT ligand rows (one ligand 1 per column);
                # column-split across DVE/ACT to balance engine load
                z1 = z1_t.pop(j)
                a1 = a1P.tile([H, CAP], BF16, tag="a1", name=f"a1_{j}")
                nc.vector.tensor_scalar(out=a1[:, 0:SPL], in0=z1[:, 0:SPL],
                                        scalar1=0.0, scalar2=0.0,
                                        op0=ALU.max, op1=ALU.add)
                nc.scalar.activation(out=a1[:, SPL:CAP], in_=z1[:, SPL:CAP],
                                     func=AF.Relu, bias=0.0, scale=1.0)
                a1_t[j] = a1

            def s_z2(j):
                z2 = psB.tile([H, CAP], F32, tag="z2", name=f"z2_{j}")
                nc.tensor.matmul(out=z2[:, :], lhsT=t["W2"][:, :],
                                 rhs=a1_t.pop(j)[:, :], start=True, stop=True)
                z2_t[j] = z2

            def s_relu2(j):
                a2 = a2P.tile([H, CAP], BF16, tag="a2", name=f"a2_{j}")
                nc.scalar.activation(out=a2[:, :], in_=z2_t.pop(j)[:, :],
                                     func=AF.Relu, bias=t["b2"][:, :], scale=1.0)
                a2_t[j] = a2

            def s_z3(j):
                z3 = psC.tile([H, CAP], F32, tag="z3", name=f"z3_{j}")
                nc.tensor.matmul(out=z3[:, :], lhsT=t["W3"][:, :],
                                 rhs=a2_t.pop(j)[:, :], start=True, stop=True)
                z3_t[j] = z3

            def s_relu3(j):
                # DVE stt: out = max(z3 + b3, zeros); accum_out = sum(out)
                a3 = a3P.tile([H, CAP], BF16, tag="a3", name=f"a3_{j}")
                nc.vector.scalar_tensor_tensor(out=a3[:, :], in0=z3_t.pop(j)[:, :],
                                               scalar=t["b3"][:, :],
                                               in1=zeros[:, :],
                                               op0=ALU.add, op1=ALU.max,
                                               accum_out=acc[:, j:j + 1])

            def step_fns(step):
                if step % CHUNK == 0 and 2 < step // CHUNK + 2 < NCH:
                    s_dma(step // CHUNK + 2)
                for off, fn in ((CHUNK, s_z1), (CHUNK + 1, s_relu1),
                                (CHUNK + 2, s_z2), (CHUNK + 3, s_relu2),
                                (CHUNK + 4, s_z3), (CHUNK + 5, s_relu3)):
                    j = step - off
                    if 0 <= j < NT:
                        fn(j)

            for step in range(NT + CHUNK + 6):
                step_fns(step)

            # ---- tail: ship per-tile sums; host reassembles per complex ----
            nc.sync.dma_start(out=acc_ap, in_=acc[:, :])

    nc.compile()
    return nc


def _get_nc(NT):
    if NT not in _CACHE:
        _CACHE[NT] = _build_nc(NT)
    return _CACHE[NT]


def kernel(protein_pos, ligand_pos, prot_emb, lig_emb,
           W1, b1, W2, b2, W3, b3, Wr1, br1, Wr2, br2,
           protein_atom_type, ligand_atom_type, protein_batch, ligand_batch):
    protein_pos = np.asarray(protein_pos, dtype=np.float32).reshape(B, P, 3)
    ligand_pos = np.asarray(ligand_pos, dtype=np.float32).reshape(B, L, 3)
    prot_emb = np.asarray(prot_emb, dtype=np.float32)
    lig_emb = np.asarray(lig_emb, dtype=np.float32)
    W1 = np.asarray(W1, dtype=np.float32)
    b1 = np.asarray(b1, np.float32).reshape(H)
    W2 = np.asarray(W2, np.float32)
    b2 = np.asarray(b2, np.float32).reshape(H)
    W3 = np.asarray(W3, np.float32)
    b3 = np.asarray(b3, np.float32).reshape(H)
    Wr1 = np.asarray(Wr1, np.float32)
    br1 = np.asarray(br1, np.float32).reshape(H)
    Wr2 = np.asarray(Wr2, np.float32).reshape(H, 1)
    br2 = np.asarray(br2, np.float32).reshape(1, 1)
    ptype = np.asarray(protein_atom_type).reshape(B, P)
    ltype = np.asarray(ligand_atom_type).reshape(B, L)

    W1a, W1b, W1c = W1[0:H], W1[H:2 * H], W1[2 * H:2 * H + RB]
    centers = np.linspace(0.0, CUTOFF, RB, dtype=np.float32)

    # pad-slot constant: z1_pad = 0 (b1 rides in the ligand one-hot rows,
    # so pads get no b1) -> a1_pad = 0 -> relu chain of b2/b3 only,
    # through the same bf16 quantization as the device
    bf = lambda x: x.astype(ml_dtypes.bfloat16).astype(np.float32)
    a2p = bf(np.maximum(b2, 0.0))
    h_pad = np.maximum(a2p @ bf(W3) + b3, 0.0)

    def cut_tiles(pairs):
        """Greedy tile cuts: <=CAP pairs and <=NPW distinct protein atoms
        per tile (atoms may split across tiles)."""
        tiles = []
        i, n = 0, len(pairs)
        while i < n:
            hi = min(i + CAP, n)
            sl = pairs[i:hi]
            natoms = len(np.unique(sl[:, 0]))
            while natoms > NPW:
                last_atoms = np.unique(sl[:, 0])[NPW:]
                hi = i + int(np.searchsorted(sl[:, 0], last_atoms[0]))
                sl = pairs[i:hi]
                natoms = len(np.unique(sl[:, 0]))
            tiles.append((i, hi))
            i = hi
        return tiles

    dists, pair_lists, tile_lists = [], [], []
    all_tiles = []                       # (complex, lo, hi) pooled globally
    for b in range(B):
        diff = protein_pos[b][:, None, :] - ligand_pos[b][None, :, :]
        dist = np.sqrt((diff * diff).sum(-1, dtype=np.float32)).astype(np.float32)
        dists.append(dist)
        pairs = np.argwhere(dist < np.float32(CUTOFF))
        pair_lists.append(pairs)
        tls = cut_tiles(pairs)
        tile_lists.append(tls)
        all_tiles += [(b, lo, hi) for (lo, hi) in tls]
    # deal tiles round-robin so every core gets an equal share
    core_tiles = [all_tiles[c::N_CORES] for c in range(N_CORES)]
    NT = max(1, max(len(ct) for ct in core_tiles))
    NCH = (NT + CHUNK - 1) // CHUNK

    common = {
        "W2": W2.astype(ml_dtypes.bfloat16), "W3": W3.astype(ml_dtypes.bfloat16),
        "Wr1": Wr1, "Wr2": Wr2,
        "b2": b2.reshape(H, 1), "b3": b3.reshape(H, 1), "br2": br2,
    }

    hlWbs = [(lig_emb[ltype[b]] @ W1b + b1).astype(np.float32) for b in range(B)]
    z1_bases = [(prot_emb[ptype[b]] @ W1a).astype(np.float32) for b in range(B)]

    in_maps = []
    for c in range(N_CORES):
        lhsT = np.zeros((H, H * CHUNK * NCH), dtype=np.float32)
        rhs = np.zeros((H, CAP * CHUNK * NCH), dtype=np.float32)
        for j, (b, lo, hi) in enumerate(core_tiles[c]):
            sl = pair_lists[b][lo:hi]
            n = hi - lo
            lhsT[0:RB, H * j:H * (j + 1)] = W1c
            lhsT[64:128, H * j:H * (j + 1)] = hlWbs[b]
            if n == 0:
                continue
            atoms, widx = np.unique(sl[:, 0], return_inverse=True)
            lhsT[32:32 + len(atoms), H * j:H * (j + 1)] = z1_bases[b][atoms]
            cols = CAP * j + np.arange(n)
            dv = dists[b][sl[:, 0], sl[:, 1]]
            u = (dv[:, None] - centers[None, :]) / np.float32(WIDTH)
            rhs[0:RB, cols] = np.exp(-0.5 * u * u).astype(np.float32).T
            rhs[32 + widx, cols] = 1.0
            rhs[64 + sl[:, 1], cols] = 1.0

        m = dict(common)
        m.update({
            "lhsT": lhsT.astype(ml_dtypes.bfloat16),
            "rhs": rhs.astype(ml_dtypes.bfloat16),
        })
        in_maps.append(m)

    nc = _get_nc(NT)
    res = bass_utils.run_bass_kernel_spmd(nc, in_maps,
                                          core_ids=list(range(N_CORES)))
    # reassemble per-complex sums from per-tile columns, then the tiny
    # 2-matvec head (~0.001% of FLOPs) on host
    tots = np.zeros((B, H), dtype=np.float32)
    for c in range(N_CORES):
        acc = res.results[c]["acc"].astype(np.float32)
        for j, (b, lo, hi) in enumerate(core_tiles[c]):
            tots[b] += acc[:, j]
    out = np.empty(B, dtype=np.float32)
    for b in range(B):
        cnt = len(pair_lists[b])
        npad = CAP * len(tile_lists[b]) - cnt
        repr_ = tots[b] / max(cnt, 1.0)
        br1p = br1 - (npad / max(cnt, 1.0)) * (h_pad @ Wr1)
        sc = np.maximum(repr_ @ Wr1 + br1p, 0.0) @ Wr2 + br2[0]
        out[b] = sc[0] if cnt > 0 else 0.0
    return out


# revision 23
# speedup vs baseline: 1.0082x; 1.0082x over previous
"""DockingScorePredictor Trainium2 kernel — valid-pair compaction.

Data-parallel over complexes: 8 cores, one complex (512 protein x 64 ligand
atoms) per core.  Only pairs within the 8A cutoff (~43%) are processed:
host packs valid pairs into NT tiles of 512 slots, each tile drawing its
protein atoms from a window of <=32 atoms (an atom's pairs may split
across consecutive tiles, so NT = ceil(cnt/512) exactly).

Per tile, ONE K=128 bf16 matmul produces the whole first layer:
  rows  0:32  of lhsT = W1c          x rhs rows  0:32  = radial basis
  rows 32:64  of lhsT = z1_base[win] x rhs rows 32:64  = one-hot protein slot
  rows 64:128 of lhsT = hlWb         x rhs rows 64:128 = one-hot ligand atom
so z1 = z1_base[p] + hlWb[l] + rb@W1c in a single 512-col pass; b1 enters
via the relu1 bias port.  Then z2 = W2.T a1 and z3 = W3.T a2 (f32r):
3 matmuls x 512 cols per tile vs 5 x 512 x 64 dense tiles before.

Engine balance (per tile): PE 3 matmuls; DVE relu1 + relu2-left; ACT
relu2-right + relu3(bias b3, accum).  b1 rides in the ligand one-hot
rows of lhsT (exactly one ligand 1 per valid column), so relu1 is
bias-free.  The radial basis, one-hots, z1_base=hp@W1a and hlWb=hl@W1b
are host-precomputed per-atom/per-pair prep (<1% of the pair-MLP FLOPs,
which all stay on device).  No masking: pad slots (zero one-hots, zero
rb) contribute the constant h_pad = relu-chain(0), folded into br1 on
host.  rhs/lhsT stream in as 4-tile chunked DMAs (big packets),
everything bf16 on the first layer (rel err ~1e-3, tolerance 2e-2).
"""
import numpy as np
from contextlib import ExitStack

import ml_dtypes

import concourse.bass as bass
import concourse.bacc as bacc
import concourse.tile as tile
from concourse import mybir
from concourse import bass_utils

F32 = mybir.dt.float32
F32R = mybir.dt.float32r
BF16 = mybir.dt.bfloat16
AF = mybir.ActivationFunctionType
ALU = mybir.AluOpType

B, P, L = 8, 512, 64
H, RB = 128, 32
CUTOFF = 8.0
N_CORES = 8
CAP = 512                      # pair slots per tile
NPW = 32                       # protein-atom window per tile
CHUNK = 4                      # tiles per DMA chunk
WIDTH = 0.5 * CUTOFF / RB + 1e-8
SPL = 256                      # relu1 column split (DVE gets [0:SPL])

_CACHE = {}


def _build_nc(NT):
    nc = bacc.Bacc("TRN2", target_bir_lowering=False, debug=False,
                   num_devices=N_CORES)
    d = {}
    NCH = (NT + CHUNK - 1) // CHUNK

    def inp(name, shape, dt):
        d[name] = nc.dram_tensor(name, shape, dt, kind="ExternalInput").ap()

    inp("rhs", [H, CAP * CHUNK * NCH], BF16)   # rb rows 0:32, one-hots 32:128
    inp("lhsT", [H, H * CHUNK * NCH], BF16)
    inp("W2", [H, H], BF16)
    inp("W3", [H, H], BF16)
    inp("b2", [H, 1], F32)
    inp("b3", [H, 1], F32)

    acc_ap = nc.dram_tensor("acc", [H, NT], F32, kind="ExternalOutput").ap()

    with tile.TileContext(nc) as tc:
        with ExitStack() as ctx:
            const = ctx.enter_context(tc.tile_pool(name="const", bufs=1))
            rhsP = ctx.enter_context(tc.tile_pool(name="rhsP", bufs=3))
            lhsP = ctx.enter_context(tc.tile_pool(name="lhsP", bufs=3))
            a1P = ctx.enter_context(tc.tile_pool(name="a1P", bufs=3))
            a2P = ctx.enter_context(tc.tile_pool(name="a2P", bufs=3))
            a3P = ctx.enter_context(tc.tile_pool(name="a3P", bufs=3))
            psA = ctx.enter_context(tc.tile_pool(name="psA", bufs=3, space="PSUM"))
            psB = ctx.enter_context(tc.tile_pool(name="psB", bufs=2, space="PSUM"))
            psC = ctx.enter_context(tc.tile_pool(name="psC", bufs=3, space="PSUM"))

            rhs_c, lhs_c = {}, {}
            z1_t, z2_t, z3_t, a1_t, a2_t = {}, {}, {}, {}, {}

            def s_dma(g, split_first=False):
                rhs = rhsP.tile([H, CAP * CHUNK], BF16, tag="rhs", name=f"rhs{g}")
                lhs = lhsP.tile([H, H * CHUNK], BF16, tag="lhs", name=f"lhs{g}")
                base_r, base_l = CAP * CHUNK * g, H * CHUNK * g
                # chunks 2-3 ride the (fill-idle) scalar engine's DMA queue so
                # their transfers overlap chunk 0/1 + consts on the sync queue
                deng = nc.scalar if g in (2, 3) else nc.sync
                if split_first:
                    # tile 0 slices land first so z1(0) starts ~1us in
                    nc.sync.dma_start(out=rhs[:, 0:CAP],
                                      in_=d["rhs"][:, base_r:base_r + CAP])
                    nc.gpsimd.dma_start(out=lhs[:, 0:H],
                                        in_=d["lhsT"][:, base_l:base_l + H])
                    nc.sync.dma_start(out=rhs[:, CAP:],
                                      in_=d["rhs"][:, base_r + CAP:base_r + CAP * CHUNK])
                    nc.gpsimd.dma_start(out=lhs[:, H:],
                                        in_=d["lhsT"][:, base_l + H:base_l + H * CHUNK])
                else:
                    deng.dma_start(out=rhs[:, :],
                                   in_=d["rhs"][:, base_r:base_r + CAP * CHUNK])
                    nc.gpsimd.dma_start(out=lhs[:, :],
                                        in_=d["lhsT"][:, base_l:base_l + H * CHUNK])
                rhs_c[g], lhs_c[g] = rhs, lhs

            s_dma(0, split_first=True)
            if NCH > 1:
                s_dma(1)
            t = {}
            for name, shape, dt in [
                ("b2", [H, 1], F32), ("b3", [H, 1], F32),
                ("W2", [H, H], BF16), ("W3", [H, H], BF16),
            ]:
                t[name] = const.tile(shape, dt, tag=name, name=name)
                nc.sync.dma_start(out=t[name], in_=d[name])

            acc = const.tile([H, NT], F32, tag="acc", name="acc")
            zeros = const.tile([H, CAP], F32, tag="zeros", name="zeros")
            nc.vector.memset(zeros[:, :], 0.0)

            # PE warmup: three fp32 matmuls (~1us+ each at cold clock) plus
            # the immediately-following z1 stream span the 3.4us HAM activity
            # window, so real matmuls run at 2.4 GHz from tile 1 on
            for w in range(3):
                wps = psC.tile([H, CAP], F32, tag="z3", name=f"warm{w}")
                nc.tensor.matmul(out=wps[:, :], lhsT=zeros[:, 0:H],
                                 rhs=zeros[:, :], start=True, stop=True)

            def s_z1(j):
                g, s = divmod(j, CHUNK)
                z1 = psA.tile([H, CAP], F32, tag="z1", name=f"z1_{j}")
                nc.tensor.matmul(out=z1[:, :],
                                 lhsT=lhs_c[g][:, H * s:H * (s + 1)],
                                 rhs=rhs_c[g][:, CAP * s:CAP * (s + 1)],
                                 start=True, stop=True)
                z1_t[j] = z1
                if s == CHUNK - 1:
                    rhs_c.pop(g), lhs_c.pop(g)

            def s_relu1(j):
                # b1 folded into lhsT ligand rows (one ligand 1 per column);
                # column-split across DVE/ACT to balance engine load
                z1 = z1_t.pop(j)
                a1 = a1P.tile([H, CAP], BF16, tag="a1", name=f"a1_{j}")
                nc.vector.tensor_scalar(out=a1[:, 0:SPL], in0=z1[:, 0:SPL],
                                        scalar1=0.0, scalar2=0.0,
                                        op0=ALU.max, op1=ALU.add)
                nc.scalar.activation(out=a1[:, SPL:CAP], in_=z1[:, SPL:CAP],
                                     func=AF.Relu, bias=0.0, scale=1.0)
                a1_t[j] = a1

            def s_z2(j):
                z2 = psB.tile([H, CAP], F32, tag="z2", name=f"z2_{j}")
                nc.tensor.matmul(out=z2[:, :], lhsT=t["W2"][:, :],
                                 rhs=a1_t.pop(j)[:, :], start=True, stop=True)
                z2_t[j] = z2

            def s_relu2(j):
                a2 = a2P.tile([H, CAP], BF16, tag="a2", name=f"a2_{j}")
                nc.scalar.activation(out=a2[:, :], in_=z2_t.pop(j)[:, :],
                                     func=AF.Relu, bias=t["b2"][:, :], scale=1.0)
                a2_t[j] = a2

            def s_z3(j):
                z3 = psC.tile([H, CAP], F32, tag="z3", name=f"z3_{j}")
                nc.tensor.matmul(out=z3[:, :], lhsT=t["W3"][:, :],
                                 rhs=a2_t.pop(j)[:, :], start=True, stop=True)
                z3_t[j] = z3

            def s_relu3(j):
                # DVE stt: out = max(z3 + b3, zeros); accum_out = sum(out)
                a3 = a3P.tile([H, CAP], BF16, tag="a3", name=f"a3_{j}")
                nc.vector.scalar_tensor_tensor(out=a3[:, :], in0=z3_t.pop(j)[:, :],
                                               scalar=t["b3"][:, :],
                                               in1=zeros[:, :],
                                               op0=ALU.add, op1=ALU.max,
                                               accum_out=acc[:, j:j + 1])

            def step_fns(step):
                if step % CHUNK == 0 and 1 < step // CHUNK + 1 < NCH:
                    s_dma(step // CHUNK + 1)
                for off, fn in ((CHUNK, s_z1), (CHUNK + 1, s_relu1),
                                (CHUNK + 2, s_z2), (CHUNK + 3, s_relu2),
                                (CHUNK + 4, s_z3), (CHUNK + 5, s_relu3)):
                    j = step - off
                    if 0 <= j < NT:
                        fn(j)

            for step in range(NT + CHUNK + 6):
                step_fns(step)

            # ---- tail: ship per-tile sums; host reassembles per complex ----
            nc.sync.dma_start(out=acc_ap, in_=acc[:, :])

    nc.compile()
    return nc


def _get_nc(NT):
    if NT not in _CACHE:
        _CACHE[NT] = _build_nc(NT)
    return _CACHE[NT]


def kernel(protein_pos, ligand_pos, prot_emb, lig_emb,
           W1, b1, W2, b2, W3, b3, Wr1, br1, Wr2, br2,
           protein_atom_type, ligand_atom_type, protein_batch, ligand_batch):
    protein_pos = np.asarray(protein_pos, dtype=np.float32).reshape(B, P, 3)
    ligand_pos = np.asarray(ligand_pos, dtype=np.float32).reshape(B, L, 3)
    prot_emb = np.asarray(prot_emb, dtype=np.float32)
    lig_emb = np.asarray(lig_emb, dtype=np.float32)
    W1 = np.asarray(W1, dtype=np.float32)
    b1 = np.asarray(b1, np.float32).reshape(H)
    W2 = np.asarray(W2, np.float32)
    b2 = np.asarray(b2, np.float32).reshape(H)
    W3 = np.asarray(W3, np.float32)
    b3 = np.asarray(b3, np.float32).reshape(H)
    Wr1 = np.asarray(Wr1, np.float32)
    br1 = np.asarray(br1, np.float32).reshape(H)
    Wr2 = np.asarray(Wr2, np.float32).reshape(H, 1)
    br2 = np.asarray(br2, np.float32).reshape(1, 1)
    ptype = np.asarray(protein_atom_type).reshape(B, P)
    ltype = np.asarray(ligand_atom_type).reshape(B, L)

    W1a, W1b, W1c = W1[0:H], W1[H:2 * H], W1[2 * H:2 * H + RB]
    centers = np.linspace(0.0, CUTOFF, RB, dtype=np.float32)

    # pad-slot constant: z1_pad = 0 (b1 rides in the ligand one-hot rows,
    # so pads get no b1) -> a1_pad = 0 -> relu chain of b2/b3 only,
    # through the same bf16 quantization as the device
    bf = lambda x: x.astype(ml_dtypes.bfloat16).astype(np.float32)
    a2p = bf(np.maximum(b2, 0.0))
    h_pad = np.maximum(a2p @ bf(W3) + b3, 0.0)

    def cut_tiles(pairs):
        """Greedy tile cuts: <=CAP pairs and <=NPW distinct protein atoms
        per tile (atoms may split across tiles)."""
        tiles = []
        i, n = 0, len(pairs)
        while i < n:
            hi = min(i + CAP, n)
            sl = pairs[i:hi]
            natoms = len(np.unique(sl[:, 0]))
            while natoms > NPW:
                last_atoms = np.unique(sl[:, 0])[NPW:]
                hi = i + int(np.searchsorted(sl[:, 0], last_atoms[0]))
                sl = pairs[i:hi]
                natoms = len(np.unique(sl[:, 0]))
            tiles.append((i, hi))
            i = hi
        return tiles

    dists, pair_lists, tile_lists = [], [], []
    all_tiles = []                       # (complex, lo, hi) pooled globally
    for b in range(B):
        diff = protein_pos[b][:, None, :] - ligand_pos[b][None, :, :]
        dist = np.sqrt((diff * diff).sum(-1, dtype=np.float32)).astype(np.float32)
        dists.append(dist)
        pairs = np.argwhere(dist < np.float32(CUTOFF))
        pair_lists.append(pairs)
        tls = cut_tiles(pairs)
        tile_lists.append(tls)
        all_tiles += [(b, lo, hi) for (lo, hi) in tls]
    # deal tiles round-robin so every core gets an equal share
    core_tiles = [all_tiles[c::N_CORES] for c in range(N_CORES)]
    NT = max(1, max(len(ct) for ct in core_tiles))
    NCH = (NT + CHUNK - 1) // CHUNK

    common = {
        "W2": W2.astype(ml_dtypes.bfloat16), "W3": W3.astype(ml_dtypes.bfloat16),
        "Wr1": Wr1, "Wr2": Wr2,
        "b2": b2.reshape(H, 1), "b3": b3.reshape(H, 1), "br2": br2,
    }

    hlWbs = [(lig_emb[ltype[b]] @ W1b + b1).astype(np.float32) for b in range(B)]
    z1_bases = [(prot_emb[ptype[b]] @ W1a).astype(np.float32) for b in range(B)]

    in_maps = []
    for c in range(N_CORES):
        lhsT = np.zeros((H, H * CHUNK * NCH), dtype=np.float32)
        rhs = np.zeros((H, CAP * CHUNK * NCH), dtype=np.float32)
        for j, (b, lo, hi) in enumerate(core_tiles[c]):
            sl = pair_lists[b][lo:hi]
            n = hi - lo
            lhsT[0:RB, H * j:H * (j + 1)] = W1c
            lhsT[64:128, H * j:H * (j + 1)] = hlWbs[b]
            if n == 0:
                continue
            atoms, widx = np.unique(sl[:, 0], return_inverse=True)
            lhsT[32:32 + len(atoms), H * j:H * (j + 1)] = z1_bases[b][atoms]
            cols = CAP * j + np.arange(n)
            dv = dists[b][sl[:, 0], sl[:, 1]]
            u = (dv[:, None] - centers[None, :]) / np.float32(WIDTH)
            rhs[0:RB, cols] = np.exp(-0.5 * u * u).astype(np.float32).T
            rhs[32 + widx, cols] = 1.0
            rhs[64 + sl[:, 1], cols] = 1.0

        m = dict(common)
        m.update({
            "lhsT": lhsT.astype(ml_dtypes.bfloat16),
            "rhs": rhs.astype(ml_dtypes.bfloat16),
        })
        in_maps.append(m)

    nc = _get_nc(NT)
    res = bass_utils.run_bass_kernel_spmd(nc, in_maps,
                                          core_ids=list(range(N_CORES)))
    # reassemble per-complex sums from per-tile columns, then the tiny
    # 2-matvec head (~0.001% of FLOPs) on host
    tots = np.zeros((B, H), dtype=np.float32)
    for c in range(N_CORES):
        acc = res.results[c]["acc"].astype(np.float32)
        for j, (b, lo, hi) in enumerate(core_tiles[c]):
            tots[b] += acc[:, j]
    out = np.empty(B, dtype=np.float32)
    for b in range(B):
        cnt = len(pair_lists[b])
        npad = CAP * len(tile_lists[b]) - cnt
        repr_ = tots[b] / max(cnt, 1.0)
        br1p = br1 - (npad / max(cnt, 1.0)) * (h_pad @ Wr1)
        sc = np.maximum(repr_ @ Wr1 + br1p, 0.0) @ Wr2 + br2[0]
        out[b] = sc[0] if cnt > 0 else 0.0
    return out
